# revision 1
# baseline (speedup 1.0000x reference)
"""
DLI loss kernel for Trainium2 (8 NeuronCores, pure data parallel over batch).

Math
----
The reference computes, per (b, j) window pair:
    logits[b,j,k] = h_last[b,j]@w_h + cterm[b,k] + fc_b
    loss_pair     = LSE_k(logits masked to k in [j+3, len_b)) - logits[b,j,j+3]
The h_last@w_h and fc_b terms are constant in k, so they cancel exactly
between the LSE and the positive logit.  The whole LSTM drops out and

    loss = sum_{b, s in [3, len_b)} [ log(sum_{k=s}^{len_b-1} e^{cterm[b,k]})
                                      - cterm[b,s] ] / sum_b (len_b - 3)
    cterm[b,k] = encoder_output[b,k,:] @ fc_w[0, H:]   (valid region only)

cterm values are O(+-2) so no max-subtraction is needed for a stable exp.

Device pipeline (per core, 16 batch rows)
-----------------------------------------
  - DMA each row b as a [128, 512] tile (2KB contiguous per partition line);
    partition p holds t in {4p..4p+3}.
  - 4 PE transposes per row -> [e, t'] layout in PSUM, one [128,512] copy to
    SBUF (alternating DVE/ACT).
  - Per-row PE matvec (fp32r, N=512) with a one-hot-expanded weight matrix
    accumulates cterm for all 16 rows into one PSUM tile [16, 512].
  - One DVE copy un-permutes and time-reverses cterm, then exp (ACT),
    mask multiply, tensor_tensor_scan (suffix sums), Ln(x+1) with accumulate,
    masked-cterm accumulate, and a tiny PE reduction to [numer, denom].
  - Host sums the 8 per-core [numer, denom] pairs and divides.
"""

import numpy as np

import concourse.bacc as bacc
import concourse.bass as bass
import concourse.mybir as mybir
import concourse.tile as tile
from concourse import masks
from concourse._compat import with_exitstack
from concourse.bass_utils import run_bass_kernel_spmd

B, T, E, H = 128, 512, 128, 128
NCORES = 8
BPC = B // NCORES  # batch rows per core

f32 = mybir.dt.float32
f32r = mybir.dt.float32r
bf16 = mybir.dt.bfloat16
i32 = mybir.dt.int32


@with_exitstack
def _dli_body(ctx, tc):
    nc = tc.nc

    enc = nc.dram_tensor("enc", [BPC, T, E], f32, kind="ExternalInput").ap()
    msk = nc.dram_tensor("mask", [BPC, T], i32, kind="ExternalInput").ap()
    wv = nc.dram_tensor("w", [E], f32, kind="ExternalInput").ap()
    out = nc.dram_tensor("out", [2], f32, kind="ExternalOutput").ap()

    const_pool = ctx.enter_context(tc.tile_pool(name="const", bufs=1))
    chunk_pool = ctx.enter_context(tc.tile_pool(name="chunk", bufs=8))
    cast_pool = ctx.enter_context(tc.tile_pool(name="cast", bufs=8))
    t4_pool = ctx.enter_context(tc.tile_pool(name="t4", bufs=6))
    tp_psum = ctx.enter_context(tc.tile_pool(name="tp_psum", bufs=4, space="PSUM"))
    ct_psum = ctx.enter_context(tc.tile_pool(name="ct_psum", bufs=1, space="PSUM"))
    fin_psum = ctx.enter_context(tc.tile_pool(name="fin_psum", bufs=1, space="PSUM"))
    sc_pool = ctx.enter_context(tc.tile_pool(name="scan", bufs=1))

    # Pre-issue the first enc DMAs so the Sync HWDGE queue is a pure enc
    # stream from the moment the kernel starts.
    NPRE = 8
    chunk_tiles = [None] * BPC

    def emit_enc_dma(b):
        chunk = chunk_pool.tile([128, T], f32)
        chunk_tiles[b] = chunk
        nc.sync.dma_start(
            chunk[:].rearrange("p (c e) -> p c e", c=4),
            enc[b].rearrange("(a c) e -> a c e", c=4),
        )

    for b in range(NPRE):
        emit_enc_dma(b)

    ident = const_pool.tile([128, 128], bf16)
    masks.make_identity(nc, ident[:])

    # warm the ACT exp/ln tables in the idle prologue
    warm = const_pool.tile([BPC, 1], f32)
    nc.vector.memset(warm[:], 0.0)
    nc.scalar.activation(warm[:], warm[:], mybir.ActivationFunctionType.Exp)
    nc.scalar.activation(warm[:], warm[:], mybir.ActivationFunctionType.Ln, bias=1.0)

    # w as a per-partition column, then expanded to the one-hot matvec weights:
    # woh[e, 16*b + m] = w[e] * (m == b).  Consts ride the idle Scalar HWDGE
    # queue so they neither delay the enc stream nor the first matvec.  The
    # woh fill copy is emitted later (inside the loop) so its wait on the w
    # DMA doesn't block the first chunk casts in the DVE queue.
    w_sb = const_pool.tile([128, 1], f32)
    nc.scalar.dma_start(w_sb[:, :], wv.rearrange("(p one) -> p one", one=1))
    woh = const_pool.tile([128, BPC * BPC], bf16)
    nc.vector.memset(woh[:].bitcast(mybir.dt.uint16), 0)

    msk_sb = sc_pool.tile([BPC, T], i32)
    nc.scalar.dma_start(msk_sb[:], msk[:, :])


    # cterm for all 16 rows, PSUM-accumulated; free index n maps to t = 4*(n%128) + n//128.
    # Software-pipelined loop with PE skew 2: the PE stream gets the
    # transposes for row b and the matvec for row b-2, so matvecs never wait
    # on a PSUM->SBUF copy issued the same iteration.
    cterm_ps = ct_psum.tile([BPC, T], f32)
    t4_tiles = [None] * BPC
    tp_tiles = [None] * BPC

    def emit_matvec(b):
        nc.tensor.matmul(
            cterm_ps[:, :],
            lhsT=woh[:, BPC * b : BPC * (b + 1)],
            rhs=t4_tiles[b][:],
            start=(b == 0),
            stop=(b == BPC - 1),
        )

    for b in range(BPC):
        if b + NPRE < BPC:
            emit_enc_dma(b + NPRE)
        chunk_bf = cast_pool.tile([128, T], bf16)
        nc.vector.tensor_copy(chunk_bf[:], chunk_tiles[b][:])
        if b == 2:
            # fill woh now: the w DMA has long completed, and casts 0-2 ran first
            nc.vector.tensor_copy(
                woh[:, :: BPC + 1], w_sb[:, 0:1].broadcast_to([128, BPC])
            )
        tp = tp_psum.tile([128, T], bf16)
        for j in range(4):
            nc.tensor.transpose(
                tp[:, 128 * j : 128 * (j + 1)],
                chunk_bf[:, 128 * j : 128 * (j + 1)],
                ident[:],
            )
        t4 = t4_pool.tile([128, T], bf16)
        t4_tiles[b] = t4
        # copies: 12 on ACT now; every 4th deferred to DVE two iterations
        # later (just before its matvec) so the DVE cast stream never stalls
        # waiting on fresh transposes.
        if b % 4 == 3:
            tp_tiles[b] = tp
        else:
            nc.scalar.copy(t4[:], tp[:])
        if b >= 2:
            bb = b - 2
            if bb % 4 == 3:
                nc.vector.tensor_copy(t4_tiles[bb][:], tp_tiles[bb][:])
            emit_matvec(bb)
    bb = BPC - 2
    if bb % 4 == 3:
        nc.vector.tensor_copy(t4_tiles[bb][:], tp_tiles[bb][:])
    emit_matvec(bb)
    bb = BPC - 1
    if bb % 4 == 3:
        nc.vector.tensor_copy(t4_tiles[bb][:], tp_tiles[bb][:])
    emit_matvec(bb)

    # mask -> f32, zero first 3 time steps (window starts need s >= 3).
    # Emitted after the loop so these DVE ops queue behind the chunk casts.
    maskf = sc_pool.tile([BPC, T], f32)
    nc.vector.tensor_copy(maskf[:], msk_sb[:])
    nc.vector.memset(maskf[:, 0:3], 0.0)
    mask3_rev = maskf[:, ::-1]

    # denominator: sum(mask3) = len_b - 3 — depends only on the mask, so it
    # runs during the loop, off the serial tail chain
    acc = sc_pool.tile([BPC, 2], f32)
    packed = sc_pool.tile([BPC, 2], f32)
    nc.vector.tensor_reduce(
        packed[:, 1:2], maskf[:], axis=mybir.AxisListType.X, op=mybir.AluOpType.add
    )

    # un-permute + time-reverse view of the PSUM cterm: element i reads
    # cterm[b, 511 - i].  Consumers read PSUM directly through this AP.
    cterm_rev = cterm_ps[:, :].rearrange("m (j p) -> m p j", j=4)[:, ::-1, ::-1]

    # E = exp(cterm)   (reversed-time coordinates, fused permute via the AP)
    e_sb = sc_pool.tile([BPC, T], f32)
    nc.scalar.activation(
        e_sb[:].rearrange("m (p j) -> m p j", j=4),
        cterm_rev,
        mybir.ActivationFunctionType.Exp,
    )

    # suffix sums with the mask folded into the scan:
    # state = (state + E[i]) * mask3_rev[i]  — resets to 0 across the invalid
    # prefix, accumulates sum(exp) over the valid region.
    s_sb = sc_pool.tile([BPC, T], f32)
    nc.vector.tensor_tensor_scan(
        s_sb[:], e_sb[:], mask3_rev, 0.0, mybir.AluOpType.add, mybir.AluOpType.mult
    )

    # u = (S - 1) * mask3; then ln(u + 1) = log(S) on valid, 0 on invalid
    u_sb = sc_pool.tile([BPC, T], f32)
    nc.vector.scalar_tensor_tensor(
        u_sb[:], s_sb[:], 1.0, mask3_rev,
        mybir.AluOpType.subtract, mybir.AluOpType.mult,
    )
    ln_sb = sc_pool.tile([BPC, T], f32)
    nc.scalar.activation(
        ln_sb[:], u_sb[:], mybir.ActivationFunctionType.Ln,
        bias=1.0, scale=1.0, accum_out=acc[:, 0:1],
    )
    # sum(mask3*cterm): order-free, so read the PSUM cterm unpermuted and the
    # mask through the matching permuted view.
    mc_sb = sc_pool.tile([BPC, T], f32)
    nc.vector.scalar_tensor_tensor(
        mc_sb[:].rearrange("m (j p) -> m j p", j=4),
        cterm_ps[:, :].rearrange("m (j p) -> m j p", j=4),
        0.0,
        maskf[:].rearrange("m (p j) -> m j p", j=4),
        mybir.AluOpType.add, mybir.AluOpType.mult, accum_out=acc[:, 1:2],
    )

    # packed[:,0] = numer_b = sum(ln) - sum(mask3*cterm)
    nc.vector.tensor_tensor(
        packed[:, 0:1], acc[:, 0:1], acc[:, 1:2], mybir.AluOpType.subtract
    )

    # cross-partition reduce on PE: out = packed^T @ ones = [sum numer, sum denom]
    ones = const_pool.tile([BPC, 1], f32)
    nc.vector.memset(ones[:], 1.0)
    fin = fin_psum.tile([2, 1], f32)
    nc.tensor.matmul(fin[:, :], lhsT=packed[:, :], rhs=ones[:, :], start=True, stop=True)
    out_sb = sc_pool.tile([2, 1], f32)
    nc.vector.tensor_copy(out_sb[:], fin[:, :])
    nc.sync.dma_start(out.rearrange("(p one) -> p one", one=1), out_sb[:])


_CACHED_NC = None


def _get_program():
    global _CACHED_NC
    if _CACHED_NC is None:
        nc = bacc.Bacc(
            "TRN2",
            target_bir_lowering=False,
            debug=False,
            enable_asserts=False,
        )
        with tile.TileContext(nc) as tc:
            _dli_body(tc)
        nc.compile()
        _CACHED_NC = nc
    return _CACHED_NC


def _make_in_maps(inputs):
    enc = np.ascontiguousarray(inputs["encoder_output"], dtype=np.float32)
    mask = np.ascontiguousarray(inputs["mask"], dtype=np.int32)
    w_e = np.ascontiguousarray(np.asarray(inputs["fc_w"], dtype=np.float32)[0, H:])
    return [
        {
            "enc": np.ascontiguousarray(enc[i * BPC : (i + 1) * BPC]),
            "mask": np.ascontiguousarray(mask[i * BPC : (i + 1) * BPC]),
            "w": w_e,
        }
        for i in range(NCORES)
    ]


def _finalize(results):
    numer = sum(float(r["out"][0]) for r in results)
    denom = sum(float(r["out"][1]) for r in results)
    return np.asarray(numer / denom, dtype=np.float32)


def kernel(**inputs) -> np.ndarray:
    nc = _get_program()
    res = run_bass_kernel_spmd(nc, _make_in_maps(inputs), list(range(NCORES)))
    return _finalize(res.results)



# revision 2
# speedup vs baseline: 1.2781x; 1.2781x over previous
"""
DLI loss kernel for Trainium2 (8 NeuronCores, pure data parallel over batch).

Math
----
The reference computes, per (b, j) window pair:
    logits[b,j,k] = h_last[b,j]@w_h + cterm[b,k] + fc_b
    loss_pair     = LSE_k(logits masked to k in [j+3, len_b)) - logits[b,j,j+3]
The h_last@w_h and fc_b terms are constant in k, so they cancel exactly
between the LSE and the positive logit.  The whole LSTM drops out and

    loss = sum_{b, s in [3, len_b)} [ log(sum_{k=s}^{len_b-1} e^{cterm[b,k]})
                                      - cterm[b,s] ] / sum_b (len_b - 3)
    cterm[b,k] = encoder_output[b,k,:] @ fc_w[0, H:]   (valid region only)

cterm values are O(+-2) so no max-subtraction is needed for a stable exp.

Device pipeline (per core, 16 batch rows)
-----------------------------------------
The host marshals enc into [B, E, T] layout and bf16 (the same
round-to-nearest cast the previous device pipeline applied on-chip), so
each DMA lands with E on partitions and T contiguous on the free dim:

  - 8 DMAs of 2 rows each ([128, 1024] bf16, 1 KB/partition-line runs),
    alternating the sync/scalar HWDGE queues.
  - 16 accumulating PE matvecs (one-hot-expanded fc_w as lhsT) compute
    cterm for all 16 rows into one PSUM tile [16, 512] - no transposes,
    no casts, no PSUM->SBUF round trips.
  - Tail: exp (ACT, fused time-reverse via AP), tensor_tensor_scan
    (suffix sums with mask fold), Ln(x+1) with accumulate, masked-cterm
    accumulate, and a tiny PE reduction to [numer, denom].
  - Host sums the 8 per-core [numer, denom] pairs and divides.
"""

import ml_dtypes
import numpy as np

import concourse.bacc as bacc
import concourse.bass as bass
import concourse.mybir as mybir
import concourse.tile as tile
from concourse._compat import with_exitstack
from concourse.bass_utils import run_bass_kernel_spmd

B, T, E, H = 128, 512, 128, 128
NCORES = 8
BPC = B // NCORES  # batch rows per core
RPD = 2  # rows per DMA chunk
NCHUNK = BPC // RPD

f32 = mybir.dt.float32
bf16 = mybir.dt.bfloat16
i32 = mybir.dt.int32


@with_exitstack
def _dli_body(ctx, tc):
    nc = tc.nc

    enc = nc.dram_tensor("enc", [BPC, E, T], bf16, kind="ExternalInput").ap()
    msk = nc.dram_tensor("mask", [BPC, T], i32, kind="ExternalInput").ap()
    wv = nc.dram_tensor("w", [E], f32, kind="ExternalInput").ap()
    out = nc.dram_tensor("out", [2], f32, kind="ExternalOutput").ap()

    const_pool = ctx.enter_context(tc.tile_pool(name="const", bufs=1))
    ct_psum = ctx.enter_context(tc.tile_pool(name="ct_psum", bufs=1, space="PSUM"))
    fin_psum = ctx.enter_context(tc.tile_pool(name="fin_psum", bufs=1, space="PSUM"))
    sc_pool = ctx.enter_context(tc.tile_pool(name="scan", bufs=1))

    # w and mask ride first on the (otherwise idle) scalar HWDGE queue so
    # the woh fill and the tail never wait behind the enc stream.
    w_sb = const_pool.tile([E, 1], f32)
    nc.scalar.dma_start(w_sb[:, :], wv.rearrange("(p one) -> p one", one=1))
    msk_sb = sc_pool.tile([BPC, T], i32)
    nc.scalar.dma_start(msk_sb[:], msk[:, :])

    # enc stream: 2 rows per DMA, alternating sync/scalar queues.
    enc_sb = const_pool.tile([E, BPC * T], bf16)
    for q in range(NCHUNK):
        eng = nc.sync if q % 2 == 0 else nc.scalar
        eng.dma_start(
            enc_sb[:, q * RPD * T : (q + 1) * RPD * T].rearrange(
                "e (r t) -> e r t", r=RPD
            ),
            enc[q * RPD : (q + 1) * RPD].rearrange("r e t -> e r t"),
        )

    # warm the ACT exp/ln tables during the DMA prologue
    warm = const_pool.tile([BPC, 1], f32)
    nc.vector.memset(warm[:], 0.0)
    nc.scalar.activation(warm[:], warm[:], mybir.ActivationFunctionType.Exp)
    nc.scalar.activation(warm[:], warm[:], mybir.ActivationFunctionType.Ln, bias=1.0)

    # one-hot-expanded matvec weights: woh[e, BPC*b + m] = w[e] * (m == b)
    woh = const_pool.tile([E, BPC * BPC], bf16)
    nc.vector.memset(woh[:].bitcast(mybir.dt.uint16), 0)
    nc.vector.tensor_copy(woh[:, :: BPC + 1], w_sb[:, 0:1].broadcast_to([E, BPC]))

    # cterm for all 16 rows accumulated in one PSUM tile; free index = t.
    cterm_ps = ct_psum.tile([BPC, T], f32)
    for b in range(BPC):
        nc.tensor.matmul(
            cterm_ps[:, :],
            lhsT=woh[:, BPC * b : BPC * (b + 1)],
            rhs=enc_sb[:, b * T : (b + 1) * T],
            start=(b == 0),
            stop=(b == BPC - 1),
        )

    # mask -> f32, zero first 3 time steps (window starts need s >= 3)
    maskf = sc_pool.tile([BPC, T], f32)
    nc.vector.tensor_copy(maskf[:], msk_sb[:])
    nc.vector.memset(maskf[:, 0:3], 0.0)
    mask3_rev = maskf[:, ::-1]

    # denominator: sum(mask3) = len_b - 3 (mask-only, runs during the stream)
    acc = sc_pool.tile([BPC, 2], f32)
    packed = sc_pool.tile([BPC, 2], f32)
    nc.vector.tensor_reduce(
        packed[:, 1:2], maskf[:], axis=mybir.AxisListType.X, op=mybir.AluOpType.add
    )

    # E = exp(cterm), time-reversed via the read AP
    e_sb = sc_pool.tile([BPC, T], f32)
    nc.scalar.activation(
        e_sb[:], cterm_ps[:, ::-1], mybir.ActivationFunctionType.Exp
    )

    # sum(mask3*cterm): needs only the matvec results, so it runs on DVE
    # in parallel with the ACT exp.
    mc_sb = sc_pool.tile([BPC, T], f32)
    nc.vector.scalar_tensor_tensor(
        mc_sb[:], cterm_ps[:, :], 0.0, maskf[:],
        mybir.AluOpType.add, mybir.AluOpType.mult, accum_out=acc[:, 1:2],
    )

    # suffix sums with the mask folded into the scan:
    # state = (E[i] + state) * mask3_rev[i] - resets across the invalid
    # tail, accumulates sum(exp) over the valid region.
    s_sb = sc_pool.tile([BPC, T], f32)
    nc.vector.tensor_tensor_scan(
        s_sb[:], e_sb[:], mask3_rev, 0.0, mybir.AluOpType.add, mybir.AluOpType.mult
    )

    # u = (S - 1) * mask3; then ln(u + 1) = log(S) on valid, 0 on invalid
    u_sb = sc_pool.tile([BPC, T], f32)
    nc.vector.scalar_tensor_tensor(
        u_sb[:], s_sb[:], 1.0, mask3_rev,
        mybir.AluOpType.subtract, mybir.AluOpType.mult,
    )
    ln_sb = sc_pool.tile([BPC, T], f32)
    nc.scalar.activation(
        ln_sb[:], u_sb[:], mybir.ActivationFunctionType.Ln,
        bias=1.0, scale=1.0, accum_out=acc[:, 0:1],
    )

    # packed[:,0] = numer_b = sum(ln) - sum(mask3*cterm)
    nc.vector.tensor_tensor(
        packed[:, 0:1], acc[:, 0:1], acc[:, 1:2], mybir.AluOpType.subtract
    )

    # cross-partition reduce on PE: out = packed^T @ ones = [sum numer, sum denom]
    ones = const_pool.tile([BPC, 1], f32)
    nc.vector.memset(ones[:], 1.0)
    fin = fin_psum.tile([2, 1], f32)
    nc.tensor.matmul(fin[:, :], lhsT=packed[:, :], rhs=ones[:, :], start=True, stop=True)
    out_sb = sc_pool.tile([2, 1], f32)
    nc.vector.tensor_copy(out_sb[:], fin[:, :])
    nc.scalar.dma_start(out.rearrange("(p one) -> p one", one=1), out_sb[:])


_CACHED_NC = None


def _get_program():
    global _CACHED_NC
    if _CACHED_NC is None:
        nc = bacc.Bacc(
            "TRN2",
            target_bir_lowering=False,
            debug=False,
            enable_asserts=False,
        )
        with tile.TileContext(nc) as tc:
            _dli_body(tc)
        nc.compile()
        _CACHED_NC = nc
    return _CACHED_NC


def _make_in_maps(inputs):
    enc = np.asarray(inputs["encoder_output"], dtype=np.float32)
    # [B, T, E] -> [B, E, T], bf16 (same RNE cast the device applied before)
    enc_t = np.ascontiguousarray(enc.transpose(0, 2, 1)).astype(ml_dtypes.bfloat16)
    mask = np.ascontiguousarray(inputs["mask"], dtype=np.int32)
    w_e = np.ascontiguousarray(np.asarray(inputs["fc_w"], dtype=np.float32)[0, H:])
    return [
        {
            "enc": np.ascontiguousarray(enc_t[i * BPC : (i + 1) * BPC]),
            "mask": np.ascontiguousarray(mask[i * BPC : (i + 1) * BPC]),
            "w": w_e,
        }
        for i in range(NCORES)
    ]


def _finalize(results):
    numer = sum(float(r["out"][0]) for r in results)
    denom = sum(float(r["out"][1]) for r in results)
    return np.asarray(numer / denom, dtype=np.float32)


def kernel(**inputs) -> np.ndarray:
    nc = _get_program()
    res = run_bass_kernel_spmd(nc, _make_in_maps(inputs), list(range(NCORES)))
    return _finalize(res.results)
